# revision 1
# baseline (speedup 1.0000x reference)
"""Trainium2 Bass kernel for nn_Explainer (gnn_message_passing).

Math (reference):
  f12[i*n+j] = concat(embed[i], embed[j]);  h = relu(f12 @ W1 + b1)
  log_alpha = h @ W2 + b2
  gate = sigmoid((log(u) - log(1-u) + log_alpha) / beta)
  sym = (gate + gate.T)/2 ; masked = adj * sym
  hg = relu((masked @ x) @ Wg1); pooled = hg.mean(0); softmax(pooled @ Wg2)

Key decomposition: f12 @ W1 + b1 = A[i] + B[j] with
  A = embed @ W1[:64] + b1   (per-row), B = embed @ W1[64:]
so log_alpha[i,j] = W2 . relu(A[i] + B[j]) + b2 -- no [N^2,128] matmul needed.

Sharding: row-blocks of the i dimension across 8 cores. The gate matrix
column-block needed for symmetrization is exchanged with an AllToAll of
[128,128] blocks (rank-symmetric, so the SPMD program is identical on all
cores). Pooled partials are combined with a tiny AllGather.
"""
import numpy as np

import concourse.bass as bass
import concourse.bacc as bacc
import concourse.tile as tile
from concourse import mybir
from concourse.bass_utils import run_bass_kernel_spmd

N = 1024
NC = 8
R = N // NC          # 128 rows per core
D = 64               # embed dim
H = 64               # hidden
F = 128              # x features
C = 8                # classes
NPAIR = R // 2       # 64 i-pairs per core
GRP = 16             # pairs per PE column-group (32 cols / 2)

F32 = mybir.dt.float32
BF16 = mybir.dt.bfloat16

# dtype used for the edge-MLP reduction matmul stream (PE runs 4x slower on f32)
MM_DT = BF16
USE_TILE_POSITION = True
DEBUG_OUTPUTS = False


def _mask_w2_np():
    """[128, NPAIR, 32] mask: 1.0 where the block-diag W2 stack has W2 values.

    Pair t -> column group g=t//16 (tile_position=(0,32g)), slot s=t%16.
    lhsT_t = W2S[:, t, :]: col 2s rows 0:64 = W2, col 2s+1 rows 64:128 = W2.
    psum out row for pair t = 32g + 2s (+1) = 2t (+1) = local i'.
    """
    cols = 32
    m = np.zeros((128, NPAIR, cols), np.float32)
    for t in range(NPAIR):
        s = t % GRP
        m[0:64, t, 2 * s] = 1.0
        m[64:128, t, 2 * s + 1] = 1.0
    return m


def _mask_w2_full_np():
    """[128, NPAIR, 128] mask for the no-tile_position fallback."""
    m = np.zeros((128, NPAIR, 128), np.float32)
    for t in range(NPAIR):
        m[0:64, t, 2 * t] = 1.0
        m[64:128, t, 2 * t + 1] = 1.0
    return m


def build():
    nc = bacc.Bacc("TRN2", target_bir_lowering=False, debug=False, num_devices=NC)

    # ---- kernel I/O ----
    # embT_full = embed.T (layout prep on host); embT_slab = embed[block].T
    # adjcol_slab = adj[:, block]  (column slab, row-major)
    embT_in = nc.dram_tensor("embT_in", [D, N], F32, kind="ExternalInput")
    embTs_in = nc.dram_tensor("embTs_in", [D, R], F32, kind="ExternalInput")
    x_full = nc.dram_tensor("x_full", [N, F], F32, kind="ExternalInput")
    adjcol_in = nc.dram_tensor("adjcol_in", [N, R], F32, kind="ExternalInput")
    noise_slab = nc.dram_tensor("noise_slab", [R, N], F32, kind="ExternalInput")
    tmp_in = nc.dram_tensor("tmp_in", [1, 1], F32, kind="ExternalInput")
    w1_in = nc.dram_tensor("w1_in", [2 * D, H], F32, kind="ExternalInput")
    b1_in = nc.dram_tensor("b1_in", [1, H], F32, kind="ExternalInput")
    w2_in = nc.dram_tensor("w2_in", [H, 1], F32, kind="ExternalInput")
    b2_in = nc.dram_tensor("b2_in", [1, 1], F32, kind="ExternalInput")
    wg1_in = nc.dram_tensor("wg1_in", [F, H], F32, kind="ExternalInput")
    wg2_in = nc.dram_tensor("wg2_in", [H, C], F32, kind="ExternalInput")
    out_dram = nc.dram_tensor("out", [1, C], F32, kind="ExternalOutput")
    dbg = {}
    if DEBUG_OUTPUTS:
        for nm, shp in [("d_la", [R, N]), ("d_gate", [R, N]),
                        ("d_mT", [128, N]), ("d_tT", [128, 128]),
                        ("d_pooled", [1, H]), ("d_ats", [H, R]),
                        ("d_hgT", [H, 128]),
                        ("d_bt", [128, N]), ("d_gcolT", [128, N])]:
            dbg[nm] = nc.dram_tensor(nm, shp, F32, kind="ExternalOutput")

    # ---- compile-time constants ----
    ident_c = nc.inline_tensor(np.eye(128, dtype=np.float32), name="identc")
    if USE_TILE_POSITION:
        maskw2_c = nc.inline_tensor(_mask_w2_np().astype(
            np.float32).astype(mybir.dt.np(MM_DT)), name="maskw2")
        w2s_cols = 32
    else:
        maskw2_c = nc.inline_tensor(_mask_w2_full_np().astype(
            np.float32).astype(mybir.dt.np(MM_DT)), name="maskw2")
        w2s_cols = 128
    ones8_c = nc.inline_tensor(np.ones((8, 1), np.float32), name="ones8")
    ones128_c = nc.inline_tensor(np.ones((1, 128), np.float32), name="ones128")

    with tile.TileContext(nc) as tc:
        with (
            tc.tile_pool(name="const", bufs=1) as constp,
            tc.tile_pool(name="big", bufs=1) as big,
            tc.tile_pool(name="tmpp", bufs=4) as tmpp,
            tc.tile_pool(name="pla", bufs=1, space="PSUM") as pla,
            tc.tile_pool(name="ptp", bufs=2, space="PSUM") as ptp,
            tc.tile_pool(name="psm", bufs=2, space="PSUM") as psm,
            tc.tile_pool(name="dram", bufs=1, space="DRAM") as dram,
        ):
            # ================= phase 0: loads + small precompute ============
            # PE warm-up: dependency-free dummy matmuls so the HAM clock-gate
            # opens (1.2 -> 2.4 GHz) before the real matmul stream starts.
            # Reads uninitialized SBUF, writes a scratch psum slot that is
            # never read.
            warm_sb = tmpp.tile([128, 512], MM_DT, tag="warm")
            nc.vector.memset(warm_sb[:], 0.0)
            for _ in range(14):
                warm_ps = psm.tile([1, 512], F32, tag="sm", name="warm_ps")
                nc.tensor.matmul(warm_ps[:], warm_sb[:, 0:1], warm_sb[:])

            # early cross-core sync on the CC queue: forces the rendezvous
            # (and absorbs PJRT launch skew) before the real AllToAll, which
            # then runs at full mesh rate.
            sync_in = dram.tile([1, 8], F32)
            sync_out = dram.tile([NC, 8], F32, addr_space="Shared")
            nc.gpsimd.dma_start(sync_in[:], b1_in[:, 0:8])
            nc.gpsimd.collective_compute(
                "AllGather", mybir.AluOpType.bypass,
                replica_groups=[list(range(NC))],
                ins=[sync_in[:].opt()], outs=[sync_out[:].opt()])

            # critical-path loads first (phase 1 prerequisites)
            embT = big.tile([D, N], F32)
            nc.sync.dma_start(embT[:], embT_in[:])
            eTs = big.tile([D, R], F32)
            nc.sync.dma_start(eTs[:], embTs_in[:])
            w1a_sb = big.tile([D, H], F32)
            nc.sync.dma_start(w1a_sb[:], w1_in[0:D, :])
            w1b_sb = big.tile([D, H], F32)
            nc.sync.dma_start(w1b_sb[:], w1_in[D:2 * D, :])
            w2_sb = big.tile([H, 1], F32)
            nc.sync.dma_start(w2_sb[:], w2_in[:])
            b1t_sb = big.tile([H, 1], F32)
            nc.sync.dma_start(b1t_sb[:], b1_in[:].rearrange("o h -> h o"))
            maskw2 = constp.tile([128, NPAIR, w2s_cols], MM_DT)
            nc.sync.dma_start(maskw2[:], maskw2_c[:])

            # remaining loads, spread across HWDGE queues so dispatch
            # doesn't serialize behind the critical loads on sync
            x_sb = big.tile([128, NC, F], F32)
            nc.sync.dma_start(
                x_sb[:], x_full[:].rearrange("(r p) f -> p r f", p=128))
            adjT = big.tile([128, NC, 128], F32)
            nc.sync.dma_start(
                adjT[:], adjcol_in[:].rearrange("(r p) b -> p r b", p=128))
            ident = constp.tile([128, 128], F32)
            nc.gpsimd.dma_start(ident[:], ident_c[:])
            ones8 = constp.tile([8, 1], F32)
            nc.gpsimd.dma_start(ones8[:], ones8_c[:])
            ones128 = constp.tile([1, 128], F32)
            nc.gpsimd.dma_start(ones128[:], ones128_c[:])
            b2_sb = big.tile([1, 1], F32)
            nc.gpsimd.dma_start(b2_sb[:], b2_in[:])
            tmp_sb = big.tile([1, 1], F32)
            nc.gpsimd.dma_start(tmp_sb[:], tmp_in[:])
            wg1_sb = big.tile([F, H], F32)
            nc.scalar.dma_start(wg1_sb[:], wg1_in[:])
            wg2_sb = big.tile([H, C], F32)
            nc.scalar.dma_start(wg2_sb[:], wg2_in[:])
            noise_sb = big.tile([R, N], F32)
            nc.scalar.dma_start(noise_sb[:], noise_slab[:])

            # x split into bf16 hi+lo for the near-fp32 bf16 GNN matmuls
            xh_bf = big.tile([128, NC, F], BF16)
            nc.vector.tensor_copy(xh_bf[:], x_sb[:])
            xl_bf = big.tile([128, NC, F], BF16)
            nc.vector.tensor_tensor(xl_bf[:], x_sb[:], xh_bf[:],
                                    op=mybir.AluOpType.subtract)

            # scaled GNN weights: 0.5 into Wg1 (symmetrize), 1/1024 into Wg2
            # (mean). scalar.mul also preloads the ACT Copy table early.
            wg1h = big.tile([F, H], F32)
            nc.scalar.mul(wg1h[:], wg1_sb[:], 0.5)
            wg2s = big.tile([H, C], F32)
            nc.vector.tensor_scalar(out=wg2s[:], in0=wg2_sb[:],
                                    scalar1=1.0 / N, scalar2=None,
                                    op0=mybir.AluOpType.mult)

            # A^T for this core's slab: [64, 128] = W1a^T @ embed_slab^T + b1
            at_ps = psm.tile([H, R], F32, tag="sm")
            nc.tensor.matmul(at_ps[:], w1a_sb[:], eTs[:])
            ats = big.tile([H, R], F32)
            nc.vector.tensor_scalar(out=ats[:], in0=at_ps[:],
                                    scalar1=b1t_sb[:], scalar2=None,
                                    op0=mybir.AluOpType.add)
            # ATstack [128, 64]: col t = bias column for pair t
            atstack = big.tile([128, NPAIR], F32)
            ats_pair = ats[:].rearrange("h (t two) -> h two t", two=2)
            nc.vector.tensor_copy(atstack[0:H, :], ats_pair[:, 0, :])
            nc.vector.tensor_copy(atstack[H:128, :], ats_pair[:, 1, :])

            # B^T (full): [64, 1024], then stacked twice -> [128, 1024] bf16
            btstack = big.tile([128, N], MM_DT)
            for jc in range(2):
                bt_ps = psm.tile([H, 512], F32, tag="sm")
                nc.tensor.matmul(bt_ps[:], w1b_sb[:],
                                 embT[:, jc * 512:(jc + 1) * 512])
                nc.vector.tensor_copy(
                    btstack[0:H, jc * 512:(jc + 1) * 512], bt_ps[:])
                nc.scalar.copy(
                    btstack[H:128, jc * 512:(jc + 1) * 512], bt_ps[:])

            # W2 stacks: maskw2 * [W2; W2] per-partition
            w2col = big.tile([128, 1], F32)
            nc.vector.tensor_copy(w2col[0:H, :], w2_sb[:])
            nc.vector.tensor_copy(w2col[H:128, :], w2_sb[:])
            w2s_t = big.tile([128, NPAIR, w2s_cols], MM_DT)
            nc.vector.tensor_scalar(
                out=w2s_t[:].rearrange("p t c -> p (t c)"),
                in0=maskw2[:].rearrange("p t c -> p (t c)"),
                scalar1=w2col[:], scalar2=None,
                op0=mybir.AluOpType.mult)

            # gate scale/bias: sigmoid(invb * pre + invb*b2)
            invb = big.tile([1, 1], F32)
            nc.vector.reciprocal(invb[:], tmp_sb[:])
            ib2 = big.tile([1, 1], F32)
            nc.vector.tensor_tensor(ib2[:], invb[:], b2_sb[:],
                                    op=mybir.AluOpType.mult)
            invb_ps = psm.tile([128, 1], F32, tag="sm")
            nc.tensor.matmul(invb_ps[:], ones128[:], invb[:])
            invb128 = big.tile([128, 1], F32)
            nc.vector.tensor_copy(invb128[:], invb_ps[:])
            ib2_ps = psm.tile([128, 1], F32, tag="sm")
            nc.tensor.matmul(ib2_ps[:], ones128[:], ib2[:])
            ib2b = big.tile([128, 1], F32)
            nc.vector.tensor_copy(ib2b[:], ib2_ps[:])

            # noise transform (overlaps phase 1 on ACT): nl = ln(u) - ln(1-u)
            logu = big.tile([R, N], F32)
            nc.scalar.activation(logu[:], noise_sb[:],
                                 mybir.ActivationFunctionType.Ln)
            log1mu = big.tile([R, N], F32)
            nc.scalar.activation(log1mu[:], noise_sb[:],
                                 mybir.ActivationFunctionType.Ln,
                                 bias=1.0, scale=-1.0)
            nl = big.tile([R, N], F32)
            nc.vector.tensor_tensor(nl[:], logu[:], log1mu[:],
                                    op=mybir.AluOpType.subtract)

            # ================= phase 1: edge MLP ============================
            # la[2t + a, j] = sum_k W2[k] relu(A[2t+a, k] + B[j, k])
            la_ps = [pla.tile([128, 512], F32, tag=f"la{jc}", name=f"la_ps{jc}")
                     for jc in range(2)]
            for t in range(NPAIR):
                g, s = t // GRP, t % GRP
                tmpb = tmpp.tile([128, N], MM_DT, tag="relu")
                if t % 5 == 2:
                    nc.scalar.activation(
                        tmpb[:], btstack[:],
                        mybir.ActivationFunctionType.Relu,
                        bias=atstack[:, t:t + 1])
                else:
                    nc.vector.tensor_scalar(
                        out=tmpb[:], in0=btstack[:],
                        scalar1=atstack[:, t:t + 1], scalar2=0.0,
                        op0=mybir.AluOpType.add, op1=mybir.AluOpType.max)
                for jc in range(2):
                    if USE_TILE_POSITION:
                        nc.tensor.matmul(
                            la_ps[jc][32 * g:32 * (g + 1), :],
                            w2s_t[:, t, :],
                            tmpb[:, jc * 512:(jc + 1) * 512],
                            start=(s == 0), stop=(s == GRP - 1),
                            tile_position=(0, 32 * g))
                    else:
                        nc.tensor.matmul(
                            la_ps[jc][:],
                            w2s_t[:, t, :],
                            tmpb[:, jc * 512:(jc + 1) * 512],
                            start=(t == 0), stop=(t == NPAIR - 1))

            la = big.tile([R, N], F32)
            for jc in range(2):
                nc.vector.tensor_copy(la[:, jc * 512:(jc + 1) * 512],
                                      la_ps[jc][:])

            # ================= phase 2: concrete gate =======================
            pre = big.tile([R, N], F32)
            nc.vector.tensor_tensor(pre[:], nl[:], la[:],
                                    op=mybir.AluOpType.add)
            gate = big.tile([R, N], F32)
            nc.scalar.activation(gate[:], pre[:],
                                 mybir.ActivationFunctionType.Sigmoid,
                                 bias=ib2b[:], scale=invb128[:])

            # ================= phase 3: exchange ===========================
            # A2A shard r (rows r*128:(r+1)*128 of input) = gate[:, rblock]
            a2a_in = dram.tile([N, R], F32)
            a2a_in_r = a2a_in[:].rearrange("(r p) b -> r p b", r=NC)
            dma_engs = [nc.sync, nc.scalar]
            for r in range(NC):
                dma_engs[r % 2].dma_start(
                    a2a_in_r[r], gate[:, r * 128:(r + 1) * 128])
            a2a_out = dram.tile([N, R], F32)
            nc.gpsimd.collective_compute(
                "AllToAll", mybir.AluOpType.bypass,
                replica_groups=[list(range(NC))],
                ins=[a2a_in[:].opt()], outs=[a2a_out[:].opt()])
            # gcolT[jl, r, i'] = gate[r*128+jl, c*128+i']
            gcolT = big.tile([128, NC, 128], F32)
            a2a_out_r = a2a_out[:].rearrange("(r p) b -> r p b", r=NC)
            for r in range(NC):
                dma_engs[r % 2].dma_start(gcolT[:, r, :], a2a_out_r[r])

            # own slab transposed: gT[jl, r, i'] = gate[c*128+i', r*128+jl]
            gTc = big.tile([128, NC, 128], F32)
            for r in range(NC):
                pt = ptp.tile([128, 128], F32, tag="tp")
                nc.tensor.transpose(pt[:], gate[:, r * 128:(r + 1) * 128],
                                    ident[:])
                nc.vector.tensor_copy(gTc[:, r, :], pt[:])

            # masked^T = (gT + gcolT) * adjT   (0.5 folded into Wg1),
            # then split into bf16 hi+lo so the t matmuls can run in bf16
            # at near-fp32 accuracy.
            mT = big.tile([128, N], F32)
            gTc_f = gTc[:].rearrange("p r b -> p (r b)")
            gcolT_f = gcolT[:].rearrange("p r b -> p (r b)")
            adjT_f = adjT[:].rearrange("p r b -> p (r b)")
            msum = big.tile([128, N], F32)
            nc.vector.tensor_tensor(msum[:], gTc_f, gcolT_f,
                                    op=mybir.AluOpType.add)
            nc.vector.tensor_tensor(mT[:], msum[:], adjT_f,
                                    op=mybir.AluOpType.mult)
            mh_bf = big.tile([128, N], BF16)
            nc.vector.tensor_copy(mh_bf[:], mT[:])
            ml_bf = big.tile([128, N], BF16)
            nc.vector.tensor_tensor(ml_bf[:], mT[:], mh_bf[:],
                                    op=mybir.AluOpType.subtract)

            # ================= phase 4: GNN =================================
            # tT[f, i'] = sum_j x[j, f] masked[i', j]
            tT_ps = pla.tile([128, 128], F32, tag="tT")
            passes = [(xh_bf, mh_bf), (xh_bf, ml_bf), (xl_bf, mh_bf)]
            for pi, (xa, mb) in enumerate(passes):
                for r in range(NC):
                    nc.tensor.matmul(
                        tT_ps[:], xa[:, r, :],
                        mb[:, r * 128:(r + 1) * 128],
                        start=(pi == 0 and r == 0),
                        stop=(pi == len(passes) - 1 and r == NC - 1))
            tT = big.tile([128, 128], F32)
            nc.vector.tensor_copy(tT[:], tT_ps[:])

            # hgT = relu(Wg1h^T @ tT): [64, 128]; pooled partial via accum
            hg_ps = psm.tile([H, 128], F32, tag="sm")
            nc.tensor.matmul(hg_ps[:], wg1h[:], tT[:])
            hgT = big.tile([H, 128], F32)
            nc.vector.tensor_scalar(out=hgT[:], in0=hg_ps[:], scalar1=0.0,
                                    scalar2=None, op0=mybir.AluOpType.max)
            pooled = big.tile([H, 1], F32)
            nc.vector.reduce_sum(pooled[:], hgT[:], axis=mybir.AxisListType.X)

            # AllGather pooled partials -> [8, 64]
            ag_in = dram.tile([1, H], F32)
            nc.gpsimd.dma_start(ag_in[:].rearrange("o h -> h o"), pooled[:])
            ag_out = dram.tile([NC, H], F32, addr_space="Shared")
            nc.gpsimd.collective_compute(
                "AllGather", mybir.AluOpType.bypass,
                replica_groups=[list(range(NC))],
                ins=[ag_in[:].opt()], outs=[ag_out[:].opt()])
            pooled8 = big.tile([NC, H], F32)
            nc.sync.dma_start(pooled8[:], ag_out[:])

            # total pooled^T [64, 1] then logits [1, C]
            poolT_ps = psm.tile([H, 1], F32, tag="sm")
            nc.tensor.matmul(poolT_ps[:], pooled8[:], ones8[:])
            poolT = big.tile([H, 1], F32)
            nc.vector.tensor_copy(poolT[:], poolT_ps[:])
            log_ps = psm.tile([1, C], F32, tag="sm")
            nc.tensor.matmul(log_ps[:], poolT[:], wg2s[:])
            z = big.tile([1, C], F32)
            nc.vector.tensor_copy(z[:], log_ps[:])

            # softmax on [1, 8] (logits are O(1): skip the max-subtraction)
            e = big.tile([1, C], F32)
            nc.scalar.activation(e[:], z[:],
                                 mybir.ActivationFunctionType.Exp)
            ssum = big.tile([1, 1], F32)
            nc.vector.reduce_sum(ssum[:], e[:], axis=mybir.AxisListType.X)
            rinv = big.tile([1, 1], F32)
            nc.vector.reciprocal(rinv[:], ssum[:])
            sm = big.tile([1, C], F32)
            nc.vector.tensor_scalar(out=sm[:], in0=e[:], scalar1=rinv[:],
                                    scalar2=None, op0=mybir.AluOpType.mult)
            nc.sync.dma_start(out_dram[:], sm[:])

            if DEBUG_OUTPUTS:
                nc.sync.dma_start(dbg["d_la"][:], la[:])
                nc.sync.dma_start(dbg["d_gate"][:], gate[:])
                nc.sync.dma_start(dbg["d_mT"][:], mT[:])
                nc.sync.dma_start(dbg["d_tT"][:], tT[:])
                nc.sync.dma_start(dbg["d_pooled"][:].rearrange("o h -> h o"),
                                  pooled[:])
                nc.sync.dma_start(dbg["d_ats"][:], ats[:])
                nc.sync.dma_start(dbg["d_hgT"][:], hgT[:])
                btf = big.tile([128, N], F32)
                nc.vector.tensor_copy(btf[:], btstack[:])
                nc.sync.dma_start(dbg["d_bt"][:], btf[:])
                nc.sync.dma_start(dbg["d_gcolT"][:],
                                  gcolT[:].rearrange("p r b -> p (r b)"))

    nc.compile()
    return nc


_NC_CACHE = None
_RUNNER_CACHE = None


def _get_nc():
    global _NC_CACHE
    if _NC_CACHE is None:
        _NC_CACHE = build()
    return _NC_CACHE


def _get_runner():
    """Cached jitted 8-core executable (run_bass_via_pjrt rebuilds the jit
    wrapper every call, costing ~300ms of host time per invocation)."""
    global _RUNNER_CACHE
    if _RUNNER_CACHE is not None:
        return _RUNNER_CACHE
    import jax
    from jax.sharding import Mesh, PartitionSpec
    from jax.experimental.shard_map import shard_map
    from concourse import bass2jax, mybir as mb
    from concourse.bass2jax import (_bass_exec_p, install_neuronx_cc_hook,
                                    partition_id_tensor)

    nc = _get_nc()
    install_neuronx_cc_hook()
    partition_name = (nc.partition_id_tensor.name
                      if nc.partition_id_tensor else None)
    in_names, out_names, out_avals, zero_outs = [], [], [], []
    for alloc in nc.m.functions[0].allocations:
        if not isinstance(alloc, mb.MemoryLocationSet):
            continue
        name = alloc.memorylocations[0].name
        if alloc.kind == "ExternalInput":
            if name == partition_name:
                continue
            in_names.append(name)
        elif alloc.kind == "ExternalOutput":
            shape = tuple(alloc.tensor_shape)
            dtype = mb.dt.np(alloc.dtype)
            out_names.append(name)
            out_avals.append(jax.core.ShapedArray(shape, dtype))
            zero_outs.append(np.zeros(shape, dtype))
    n_params = len(in_names)
    all_in = in_names + out_names
    if partition_name is not None:
        all_in = all_in + [partition_name]

    def _body(*args):
        operands = list(args)
        if partition_name is not None:
            operands.append(partition_id_tensor())
        outs = _bass_exec_p.bind(
            *operands,
            out_avals=tuple(out_avals),
            in_names=tuple(all_in),
            out_names=tuple(out_names),
            lowering_input_output_aliases=(),
            sim_require_finite=True,
            sim_require_nnan=True,
            nc=nc,
        )
        return tuple(outs)

    devices = jax.devices()[:NC]
    mesh = Mesh(np.asarray(devices), ("core",))
    n_outs = len(out_names)
    sharded = jax.jit(
        shard_map(_body, mesh=mesh,
                  in_specs=(PartitionSpec("core"),) * (n_params + n_outs),
                  out_specs=(PartitionSpec("core"),) * n_outs,
                  check_rep=False),
        donate_argnums=tuple(range(n_params, n_params + n_outs)),
        keep_unused=True)

    def run(in_maps):
        concat_in = [
            np.concatenate([np.asarray(in_maps[c][nm]) for c in range(NC)],
                           axis=0)
            for nm in in_names
        ]
        concat_zeros = [
            np.zeros((NC * z.shape[0], *z.shape[1:]), z.dtype)
            for z in zero_outs
        ]
        out_arrs = sharded(*concat_in, *concat_zeros)
        return [
            {nm: np.asarray(out_arrs[i]).reshape(NC, *out_avals[i].shape)[c]
             for i, nm in enumerate(out_names)}
            for c in range(NC)
        ]

    _RUNNER_CACHE = run
    return run


def kernel(**inputs):
    x = np.ascontiguousarray(np.asarray(inputs["x"], dtype=np.float32))
    embed = np.ascontiguousarray(np.asarray(inputs["embed"], dtype=np.float32))
    adj = np.ascontiguousarray(np.asarray(inputs["adj"], dtype=np.float32))
    tmp = np.asarray(inputs["tmp"], dtype=np.float32).reshape(1, 1)
    noise = np.asarray(inputs["noise"], dtype=np.float32).reshape(N, N)
    W1 = np.ascontiguousarray(np.asarray(inputs["W1"], dtype=np.float32))
    b1 = np.asarray(inputs["b1"], dtype=np.float32).reshape(1, H)
    W2 = np.ascontiguousarray(np.asarray(inputs["W2"], dtype=np.float32))
    b2 = np.asarray(inputs["b2"], dtype=np.float32).reshape(1, 1)
    Wg1 = np.ascontiguousarray(np.asarray(inputs["Wg1"], dtype=np.float32))
    Wg2 = np.ascontiguousarray(np.asarray(inputs["Wg2"], dtype=np.float32))

    in_maps = build_in_maps(x, embed, adj, noise, tmp, W1, b1, W2, b2, Wg1, Wg2)
    try:
        results = _get_runner()(in_maps)
        return np.asarray(results[0]["out"], dtype=np.float32).reshape(1, C)
    except Exception:
        nc = _get_nc()
        res = run_bass_kernel_spmd(nc, in_maps, core_ids=list(range(NC)))
        return np.asarray(res.results[0]["out"],
                          dtype=np.float32).reshape(1, C)


def build_in_maps(x, embed, adj, noise, tmp, W1, b1, W2, b2, Wg1, Wg2):
    embT = np.ascontiguousarray(embed.T)
    in_maps = []
    for c in range(NC):
        sl = slice(c * R, (c + 1) * R)
        in_maps.append({
            "embT_in": embT,
            "embTs_in": np.ascontiguousarray(embT[:, sl]),
            "x_full": x,
            "adjcol_in": np.ascontiguousarray(adj[sl].T),
            "noise_slab": np.ascontiguousarray(noise[sl]),
            "tmp_in": tmp,
            "w1_in": W1,
            "b1_in": b1,
            "w2_in": W2,
            "b2_in": b2,
            "wg1_in": Wg1,
            "wg2_in": Wg2,
        })
    return in_maps

